# revision 31
# baseline (speedup 1.0000x reference)
"""Detection-loss kernel for Trainium2 (8 NeuronCores, data-parallel over batch).

Reference computes: scatter 64 targets/image into a [B,C,H,W] map + mask,
then masked SmoothL1(preds, map).sum() / num_objects.

The mask is nonzero at <= B*T positions, so the loss only depends on preds
at those cells.  Each core *gathers* preds at its images' (gy,gx) cells via
one indirect DMA (256 descriptors x 28B, channels-last host relayout), runs a
5-op SmoothL1 chain, reduces across partitions with a ones-matmul on the PE
array (bf16 operands, single pass) plus one segmented row-reduce, and DMAs
out two partial scalars.  Host combines the 8 cores' partials.

Collision handling (last-writer-wins, matching jax scatter): the host
resolves winners while packing offsets; each losing target's descriptor
points at a zero pad cell appended to the preds shard and its target value
is 0, so it contributes exactly 0 loss.  num_objects comes from summing the
winner mask (shipped as 7-wide columns next to the target values so one
matmul reduces both).

Per-core layout (4 images, 2 groups of 128 targets on partitions):
  partition p in [0,128), group g in {0,1}:
    image j = g*2 + p//64 (local), target t = p%64, channel c in [0,7)
  flat gather offset = (gy*W + gx)*C + j*C*H*W  (channels-last), or the pad
  cell NELEM for losers.  smoothl1 = 2*h*(|d|-h), h = 0.5*min(|d|,1); the
  factor 2 is folded into the host-side combine.

A 2-descriptor dummy indirect DMA runs during the input-DMA flight time so
the real gather's SWDGE emission hits warm ucode paths.
"""

import numpy as np

B, C, H, W = 32, 7, 400, 400
T = 64
NCORES = 8
BLOC = B // NCORES          # 4 images per core
HW = H * W                  # 160000
CHW = C * HW                # 1120000
NELEM = BLOC * CHW          # 4480000 elements per core
PAD = 8                     # zero pad cells for loser descriptors
NG = BLOC * T // 128        # 2 groups of 128 targets
P = 128
GC = NG * C                 # 14 value columns

NUM_GATHERS = 1             # 1: single 256-desc indirect DMA; 2: one per group

_cached = {}
TRACE = False  # set True (e.g. from test.py) to capture an NTFF profile


def _build_nc():
    import concourse.bacc as bacc
    import concourse.bass as bass
    import concourse.tile as tile
    import concourse.mybir as mybir

    f32 = mybir.dt.float32
    i32 = mybir.dt.int32
    OP = mybir.AluOpType

    nc = bacc.Bacc(
        "TRN2",
        target_bir_lowering=False,
        debug=False,
        enable_asserts=False,
        num_devices=NCORES,
    )

    AX = mybir.AxisListType

    bf16 = mybir.dt.bfloat16

    preds_flat = nc.dram_tensor("preds_flat", [NELEM + PAD, 1], f32, kind="ExternalInput")
    # aux (single input DMA): [offs (2, int32) | tv (14) | wfull (14)] — float
    # payload shipped as raw int32 bits, bitcast back on device.  One DMA so
    # the offsets can't straggle behind another transfer on the same ring.
    aux_d = nc.dram_tensor("aux", [P, NG + 2 * GC], i32, kind="ExternalInput")
    out_d = nc.dram_tensor("out", [1, 2], f32, kind="ExternalOutput")

    with tile.TileContext(nc) as tc:
        with (
            tc.tile_pool(name="sbuf", bufs=1) as sb,
            tc.tile_pool(name="psum", bufs=1, space="PSUM") as pp,
        ):
            X = sb.tile([P, NG + 2 * GC], i32)
            nc.scalar.dma_start(X[:], aux_d[:, :])
            offs = X[:, 0:NG]
            A = X[:, NG : NG + 2 * GC].bitcast(f32)
            ones = sb.tile([P, 1], bf16)
            nc.vector.memset(ones[:], 1.0)

            # warm-up: a 2-descriptor dummy indirect DMA runs while the aux
            # DMA is in flight, so the real gather's Q7 emission hits warm
            # ucode paths (~100ns) instead of paying first-call setup
            offs0 = sb.tile([P, 1], i32)
            nc.gpsimd.memset(offs0[:], 0)
            warm = sb.tile([P, C], f32)
            nc.gpsimd.indirect_dma_start(
                out=warm[0:2, :],
                out_offset=None,
                in_=preds_flat[:, :],
                in_offset=bass.IndirectOffsetOnAxis(ap=offs0[0:2, 0:1], axis=0),
            )

            # gather: one 28B descriptor per target (channels-last layout)
            gat = sb.tile([P, GC], f32)
            if NUM_GATHERS == 1:
                nc.gpsimd.indirect_dma_start(
                    out=gat[:, :],
                    out_offset=None,
                    in_=preds_flat[:, :],
                    in_offset=bass.IndirectOffsetOnAxis(ap=offs, axis=0),
                )
            else:
                for g in range(NG):
                    nc.gpsimd.indirect_dma_start(
                        out=gat[:, g * C : (g + 1) * C],
                        out_offset=None,
                        in_=preds_flat[:, :],
                        in_offset=bass.IndirectOffsetOnAxis(
                            ap=offs[:, g : g + 1], axis=0
                        ),
                    )

            # bf16 staging for the matmul operands: wfull cast runs early (off
            # the critical path); le is written as bf16 by its producing op
            Ab = sb.tile([P, 2 * GC], bf16)
            nc.vector.tensor_copy(Ab[:, GC : 2 * GC], A[:, GC : 2 * GC])

            # smoothl1(d)/2 = h*(|d|-h), h = 0.5*min(|d|,1); losers have
            # d == 0 exactly (pad cell, tv 0) so they contribute 0
            d = sb.tile([P, GC], f32)
            nc.vector.tensor_sub(d[:], gat[:], A[:, 0:GC])
            ad = sb.tile([P, GC], f32)
            nc.vector.scalar_tensor_tensor(ad[:], d[:], -1.0, d[:], OP.mult, OP.max)
            h = sb.tile([P, GC], f32)
            nc.vector.tensor_scalar(h[:], ad[:], 1.0, 0.5, OP.min, OP.mult)
            u = sb.tile([P, GC], f32)
            nc.vector.tensor_sub(u[:], ad[:], h[:])
            nc.vector.tensor_mul(Ab[:, 0:GC], h[:], u[:])  # le, bf16 on write

            # partition reduction: ones^T @ [le | wfull], single-pass bf16
            ps = pp.tile([1, 2 * GC], f32)
            nc.tensor.matmul(ps[:], ones[:], Ab[:, :])
            res = sb.tile([1, 2], f32)
            nc.vector.reduce_sum(
                res[:, :],
                ps[:, :].rearrange("p (g c) -> p g c", g=2),
                axis=AX.X,
            )
            nc.sync.dma_start(out_d[:, :], res[:])

    nc.compile()
    return nc


def _get_nc():
    if "nc" not in _cached:
        _cached["nc"] = _build_nc()
    return _cached["nc"]


def _make_in_maps(preds, targets):
    # grid cells exactly as the reference computes them (all-f32 arithmetic)
    five = np.float32(W / 80.0)
    gx = np.clip(np.floor(targets[..., 0] * five), 0, W - 1).astype(np.int64)
    gy = np.clip(np.floor(targets[..., 1] * five), 0, H - 1).astype(np.int64)
    cells = gy * W + gx                                  # [B, T]
    # last-writer-wins: target t loses if any t' > t hits the same cell
    eq = cells[:, :, None] == cells[:, None, :]          # [B, T, T]
    later = np.triu(np.ones((T, T), dtype=bool), k=1)
    win = ~np.any(eq & later, axis=2)                    # [B, T]

    # channels-last relayout so each target's 7 channels are one contiguous
    # 28B indirect-DMA row; 8 zero pad cells per shard catch the losers
    preds_t = np.ascontiguousarray(preds.transpose(0, 2, 3, 1))

    jj = (np.arange(P) // 64)[:, None]                   # [128, 1]
    gg = np.arange(NG)[None, :]                          # [1, 2]
    jloc = gg * 2 + jj                                   # local image index
    tt = (np.arange(P) % 64)[:, None]                    # target index

    in_maps = []
    for k in range(NCORES):
        pshard = np.zeros((NELEM + PAD, 1), dtype=np.float32)
        pshard[:NELEM, 0] = preds_t[k * BLOC : (k + 1) * BLOC].reshape(NELEM)

        jglob = k * BLOC + jloc                          # [128, 2] global image
        wsh = win[jglob, tt]                             # [128, 2]
        offs = np.where(
            wsh, cells[jglob, tt] * C + jloc * CHW, NELEM
        ).astype(np.int32)

        tvals = targets[jglob, tt, :] * wsh[..., None]   # [128, 2, 7]
        wfull = np.broadcast_to(
            wsh[..., None].astype(np.float32), (P, NG, C)
        )

        # sort the 256 descriptors by DRAM offset (the partial sums are
        # permutation-invariant): each SDMA engine then reads a contiguous
        # address range, tightening the gather drain
        flat_off = offs.reshape(P * NG)
        order = np.argsort(flat_off, kind="stable")
        offs = flat_off[order].reshape(P, NG)
        tv_s = tvals.reshape(P * NG, C)[order].reshape(P, GC)
        wf_s = np.ascontiguousarray(wfull).reshape(P * NG, C)[order].reshape(P, GC)

        fbits = np.concatenate([tv_s, wf_s], axis=1).astype(np.float32).view(np.int32)
        aux = np.concatenate([offs, fbits], axis=1)

        in_maps.append(
            {"preds_flat": pshard, "aux": np.ascontiguousarray(aux)}
        )
    return in_maps


def kernel(preds, targets):
    from concourse.bass_utils import run_bass_kernel_spmd

    preds = np.ascontiguousarray(np.asarray(preds), dtype=np.float32)
    targets = np.ascontiguousarray(np.asarray(targets), dtype=np.float32)
    assert preds.shape == (B, C, H, W) and targets.shape == (B, T, C)

    nc = _get_nc()
    in_maps = _make_in_maps(preds, targets)
    res = run_bass_kernel_spmd(nc, in_maps, list(range(NCORES)), trace=TRACE)
    _cached["last_results"] = res

    lsum = 0.0
    nsum = 0.0
    for k in range(NCORES):
        part = res.results[k]["out"].reshape(2)
        lsum += 2.0 * float(part[0])
        nsum += float(part[1]) / 7.0
    loss = np.float32(lsum / (nsum + 1e-6))
    return loss, np.float32(nsum)


# revision 32
# speedup vs baseline: 1.0590x; 1.0590x over previous
"""Detection-loss kernel for Trainium2 (8 NeuronCores, data-parallel over batch).

Reference computes: scatter 64 targets/image into a [B,C,H,W] map + mask,
then masked SmoothL1(preds, map).sum() / num_objects.

The mask is nonzero at <= B*T positions, so the loss only depends on preds
at those cells.  Each core *gathers* preds at its images' (gy,gx) cells via
one indirect DMA (256 descriptors x 28B, channels-last host relayout), runs a
5-op SmoothL1 chain, reduces across partitions with a ones-matmul on the PE
array (bf16 operands, single pass) plus one segmented row-reduce, and DMAs
out two partial scalars.  Host combines the 8 cores' partials.

Collision handling (last-writer-wins, matching jax scatter): the host
resolves winners while packing offsets; each losing target's descriptor
points at a zero pad cell appended to the preds shard and its target value
is 0, so it contributes exactly 0 loss.  num_objects comes from summing the
winner mask (shipped as 7-wide columns next to the target values so one
matmul reduces both).

Per-core layout (4 images, 2 groups of 128 targets on partitions):
  partition p in [0,128), group g in {0,1}:
    image j = g*2 + p//64 (local), target t = p%64, channel c in [0,7)
  flat gather offset = (gy*W + gx)*C + j*C*H*W  (channels-last), or the pad
  cell NELEM for losers.  smoothl1 = 2*h*(|d|-h), h = 0.5*min(|d|,1); the
  factor 2 is folded into the host-side combine.

A 2-descriptor dummy indirect DMA runs during the input-DMA flight time so
the real gather's SWDGE emission hits warm ucode paths.
"""

import numpy as np

B, C, H, W = 32, 7, 400, 400
T = 64
NCORES = 8
BLOC = B // NCORES          # 4 images per core
HW = H * W                  # 160000
CHW = C * HW                # 1120000
NELEM = BLOC * CHW          # 4480000 elements per core
PAD = 8                     # zero pad cells for loser descriptors
NG = BLOC * T // 128        # 2 groups of 128 targets
P = 128
GC = NG * C                 # 14 value columns

NUM_GATHERS = 1             # 1: single 256-desc indirect DMA; 2: one per group

_cached = {}
TRACE = False  # set True (e.g. from test.py) to capture an NTFF profile


def _build_nc():
    import concourse.bacc as bacc
    import concourse.bass as bass
    import concourse.tile as tile
    import concourse.mybir as mybir

    f32 = mybir.dt.float32
    i32 = mybir.dt.int32
    OP = mybir.AluOpType

    nc = bacc.Bacc(
        "TRN2",
        target_bir_lowering=False,
        debug=False,
        enable_asserts=False,
        num_devices=NCORES,
    )

    AX = mybir.AxisListType

    bf16 = mybir.dt.bfloat16

    preds_flat = nc.dram_tensor("preds_flat", [NELEM + PAD, 1], f32, kind="ExternalInput")
    # aux (single input DMA): [offs (2, int32) | tv (14) | wfull (14)] — float
    # payload shipped as raw int32 bits, bitcast back on device.  One DMA so
    # the offsets can't straggle behind another transfer on the same ring.
    aux_d = nc.dram_tensor("aux", [P, NG + 2 * GC], i32, kind="ExternalInput")
    out_d = nc.dram_tensor("out", [1, 2], f32, kind="ExternalOutput")

    # raw SBUF/PSUM handles instead of tile pools: every buffer is written
    # exactly once (no WAR hazards), and skipping the pools removes their
    # open/close all-engine handshakes from the measured window
    with tile.TileContext(nc) as tc:
        with (
            nc.sbuf_tensor("X", [P, NG + 2 * GC], i32) as X,
            nc.sbuf_tensor("ones", [P, 1], bf16) as ones,
            nc.sbuf_tensor("offs0", [P, 1], i32) as offs0,
            nc.sbuf_tensor("warm", [P, C], f32) as warm,
            nc.sbuf_tensor("gat", [P, GC], f32) as gat,
            nc.sbuf_tensor("Ab", [P, 2 * GC], bf16) as Ab,
            nc.sbuf_tensor("d", [P, GC], f32) as d,
            nc.sbuf_tensor("ad", [P, GC], f32) as ad,
            nc.sbuf_tensor("h", [P, GC], f32) as h,
            nc.sbuf_tensor("u", [P, GC], f32) as u,
            nc.sbuf_tensor("res", [1, 2], f32) as res,
            nc.psum_tensor("ps", [1, 2 * GC], f32) as ps,
        ):
            nc.scalar.dma_start(X[:, :], aux_d[:, :])
            offs = X[:, 0:NG]
            A = X[:, NG : NG + 2 * GC].bitcast(f32)
            nc.vector.memset(ones[:, :], 1.0)

            # warm-up: a 2-descriptor dummy indirect DMA runs while the aux
            # DMA is in flight, so the real gather's Q7 emission hits warm
            # ucode paths (~100ns) instead of paying first-call setup
            nc.gpsimd.memset(offs0[:, :], 0)
            nc.gpsimd.indirect_dma_start(
                out=warm[0:2, :],
                out_offset=None,
                in_=preds_flat[:, :],
                in_offset=bass.IndirectOffsetOnAxis(ap=offs0[0:2, 0:1], axis=0),
            )

            # gather: one 28B descriptor per target (channels-last layout)
            nc.gpsimd.indirect_dma_start(
                out=gat[:, :],
                out_offset=None,
                in_=preds_flat[:, :],
                in_offset=bass.IndirectOffsetOnAxis(ap=offs, axis=0),
            )

            # bf16 staging for the matmul operands: wfull cast runs early (off
            # the critical path); le is written as bf16 by its producing op
            nc.vector.tensor_copy(Ab[:, GC : 2 * GC], A[:, GC : 2 * GC])

            # smoothl1(d)/2 = h*(|d|-h), h = 0.5*min(|d|,1); losers have
            # d == 0 exactly (pad cell, tv 0) so they contribute 0
            nc.vector.tensor_sub(d[:, :], gat[:, :], A[:, 0:GC])
            nc.vector.scalar_tensor_tensor(
                ad[:, :], d[:, :], -1.0, d[:, :], OP.mult, OP.max
            )
            nc.vector.tensor_scalar(h[:, :], ad[:, :], 1.0, 0.5, OP.min, OP.mult)
            nc.vector.tensor_sub(u[:, :], ad[:, :], h[:, :])
            nc.vector.tensor_mul(Ab[:, 0:GC], h[:, :], u[:, :])  # le, bf16

            # partition reduction: ones^T @ [le | wfull], single-pass bf16
            nc.tensor.matmul(ps[:, :], ones[:, :], Ab[:, :])
            nc.vector.reduce_sum(
                res[:, :],
                ps[:, :].rearrange("p (g c) -> p g c", g=2),
                axis=AX.X,
            )
            nc.sync.dma_start(out_d[:, :], res[:, :])

    nc.compile()
    return nc


def _get_nc():
    if "nc" not in _cached:
        _cached["nc"] = _build_nc()
    return _cached["nc"]


def _make_in_maps(preds, targets):
    # grid cells exactly as the reference computes them (all-f32 arithmetic)
    five = np.float32(W / 80.0)
    gx = np.clip(np.floor(targets[..., 0] * five), 0, W - 1).astype(np.int64)
    gy = np.clip(np.floor(targets[..., 1] * five), 0, H - 1).astype(np.int64)
    cells = gy * W + gx                                  # [B, T]
    # last-writer-wins: target t loses if any t' > t hits the same cell
    eq = cells[:, :, None] == cells[:, None, :]          # [B, T, T]
    later = np.triu(np.ones((T, T), dtype=bool), k=1)
    win = ~np.any(eq & later, axis=2)                    # [B, T]

    # channels-last relayout so each target's 7 channels are one contiguous
    # 28B indirect-DMA row; 8 zero pad cells per shard catch the losers
    preds_t = np.ascontiguousarray(preds.transpose(0, 2, 3, 1))

    jj = (np.arange(P) // 64)[:, None]                   # [128, 1]
    gg = np.arange(NG)[None, :]                          # [1, 2]
    jloc = gg * 2 + jj                                   # local image index
    tt = (np.arange(P) % 64)[:, None]                    # target index

    in_maps = []
    for k in range(NCORES):
        pshard = np.zeros((NELEM + PAD, 1), dtype=np.float32)
        pshard[:NELEM, 0] = preds_t[k * BLOC : (k + 1) * BLOC].reshape(NELEM)

        jglob = k * BLOC + jloc                          # [128, 2] global image
        wsh = win[jglob, tt]                             # [128, 2]
        offs = np.where(
            wsh, cells[jglob, tt] * C + jloc * CHW, NELEM
        ).astype(np.int32)

        tvals = targets[jglob, tt, :] * wsh[..., None]   # [128, 2, 7]
        wfull = np.broadcast_to(
            wsh[..., None].astype(np.float32), (P, NG, C)
        )

        # sort the 256 descriptors by DRAM offset (the partial sums are
        # permutation-invariant): each SDMA engine then reads a contiguous
        # address range, tightening the gather drain
        flat_off = offs.reshape(P * NG)
        order = np.argsort(flat_off, kind="stable")
        offs = flat_off[order].reshape(P, NG)
        tv_s = tvals.reshape(P * NG, C)[order].reshape(P, GC)
        wf_s = np.ascontiguousarray(wfull).reshape(P * NG, C)[order].reshape(P, GC)

        fbits = np.concatenate([tv_s, wf_s], axis=1).astype(np.float32).view(np.int32)
        aux = np.concatenate([offs, fbits], axis=1)

        in_maps.append(
            {"preds_flat": pshard, "aux": np.ascontiguousarray(aux)}
        )
    return in_maps


def kernel(preds, targets):
    from concourse.bass_utils import run_bass_kernel_spmd

    preds = np.ascontiguousarray(np.asarray(preds), dtype=np.float32)
    targets = np.ascontiguousarray(np.asarray(targets), dtype=np.float32)
    assert preds.shape == (B, C, H, W) and targets.shape == (B, T, C)

    nc = _get_nc()
    in_maps = _make_in_maps(preds, targets)
    res = run_bass_kernel_spmd(nc, in_maps, list(range(NCORES)), trace=TRACE)
    _cached["last_results"] = res

    lsum = 0.0
    nsum = 0.0
    for k in range(NCORES):
        part = res.results[k]["out"].reshape(2)
        lsum += 2.0 * float(part[0])
        nsum += float(part[1]) / 7.0
    loss = np.float32(lsum / (nsum + 1e-6))
    return loss, np.float32(nsum)


# revision 33
# speedup vs baseline: 1.0902x; 1.0294x over previous
"""Detection-loss kernel for Trainium2 (8 NeuronCores, data-parallel over batch).

Reference computes: scatter 64 targets/image into a [B,C,H,W] map + mask,
then masked SmoothL1(preds, map).sum() / num_objects.

The mask is nonzero at <= B*T positions, so the loss only depends on preds
at those cells.  Each core *gathers* preds at its images' (gy,gx) cells via
one indirect DMA (256 descriptors x 28B, channels-last host relayout), runs a
5-op SmoothL1 chain, reduces across partitions with a ones-matmul on the PE
array (bf16 operands, single pass) plus one segmented row-reduce, and DMAs
out two partial scalars.  Host combines the 8 cores' partials.

Collision handling (last-writer-wins, matching jax scatter): the host
resolves winners while packing offsets; each losing target's descriptor
points at a zero pad cell appended to the preds shard and its target value
is 0, so it contributes exactly 0 loss.  num_objects comes from summing the
winner mask (shipped as 7-wide columns next to the target values so one
matmul reduces both).

Per-core layout (4 images, 2 groups of 128 targets on partitions):
  partition p in [0,128), group g in {0,1}:
    image j = g*2 + p//64 (local), target t = p%64, channel c in [0,7)
  flat gather offset = (gy*W + gx)*C + j*C*H*W  (channels-last), or the pad
  cell NELEM for losers.  smoothl1 = 2*h*(|d|-h), h = 0.5*min(|d|,1); the
  factor 2 is folded into the host-side combine.

A 2-descriptor dummy indirect DMA runs during the input-DMA flight time so
the real gather's SWDGE emission hits warm ucode paths.
"""

import numpy as np

B, C, H, W = 32, 7, 400, 400
T = 64
NCORES = 8
BLOC = B // NCORES          # 4 images per core
HW = H * W                  # 160000
CHW = C * HW                # 1120000
NELEM = BLOC * CHW          # 4480000 elements per core
PAD = 8                     # zero pad cells for loser descriptors
NG = BLOC * T // 128        # 2 groups of 128 targets
P = 128
GC = NG * C                 # 14 value columns

NUM_GATHERS = 1             # 1: single 256-desc indirect DMA; 2: one per group

_cached = {}
TRACE = False  # set True (e.g. from test.py) to capture an NTFF profile


def _build_nc():
    import concourse.bacc as bacc
    import concourse.bass as bass
    import concourse.tile as tile
    import concourse.mybir as mybir

    f32 = mybir.dt.float32
    i32 = mybir.dt.int32
    OP = mybir.AluOpType

    nc = bacc.Bacc(
        "TRN2",
        target_bir_lowering=False,
        debug=False,
        enable_asserts=False,
        num_devices=NCORES,
    )

    AX = mybir.AxisListType

    bf16 = mybir.dt.bfloat16

    preds_flat = nc.dram_tensor("preds_flat", [NELEM + PAD, 1], f32, kind="ExternalInput")
    # aux (single input DMA): [offs (2, int32) | tv (14) | wfull (14)] — float
    # payload shipped as raw int32 bits, bitcast back on device.  One DMA so
    # the offsets can't straggle behind another transfer on the same ring.
    aux_d = nc.dram_tensor("aux", [P, NG + 2 * GC], i32, kind="ExternalInput")
    out_d = nc.dram_tensor("out", [1, 2], f32, kind="ExternalOutput")

    with tile.TileContext(nc) as tc:
        with (
            tc.tile_pool(name="sbuf", bufs=1) as sb,
            tc.tile_pool(name="psum", bufs=1, space="PSUM") as pp,
        ):
            X = sb.tile([P, NG + 2 * GC], i32)
            nc.scalar.dma_start(X[:], aux_d[:, :])
            offs = X[:, 0:NG]
            A = X[:, NG : NG + 2 * GC].bitcast(f32)
            ones = sb.tile([P, 1], bf16)
            nc.vector.memset(ones[:], 1.0)

            # warm-up: a 2-descriptor dummy indirect DMA runs while the aux
            # DMA is in flight, so the real gather's Q7 emission hits warm
            # ucode paths (~100ns) instead of paying first-call setup
            offs0 = sb.tile([P, 1], i32)
            nc.gpsimd.memset(offs0[:], 0)
            warm = sb.tile([P, C], f32)
            nc.gpsimd.indirect_dma_start(
                out=warm[0:2, :],
                out_offset=None,
                in_=preds_flat[:, :],
                in_offset=bass.IndirectOffsetOnAxis(ap=offs0[0:2, 0:1], axis=0),
            )

            # gather: one 28B descriptor per target (channels-last layout)
            gat = sb.tile([P, GC], f32)
            if NUM_GATHERS == 1:
                nc.gpsimd.indirect_dma_start(
                    out=gat[:, :],
                    out_offset=None,
                    in_=preds_flat[:, :],
                    in_offset=bass.IndirectOffsetOnAxis(ap=offs, axis=0),
                )
            else:
                for g in range(NG):
                    nc.gpsimd.indirect_dma_start(
                        out=gat[:, g * C : (g + 1) * C],
                        out_offset=None,
                        in_=preds_flat[:, :],
                        in_offset=bass.IndirectOffsetOnAxis(
                            ap=offs[:, g : g + 1], axis=0
                        ),
                    )

            # bf16 staging for the matmul operands: wfull cast runs early (off
            # the critical path); le is written as bf16 by its producing op
            Ab = sb.tile([P, 2 * GC], bf16)
            nc.vector.tensor_copy(Ab[:, GC : 2 * GC], A[:, GC : 2 * GC])

            # smoothl1(d)/2 = h*(|d|-h), h = 0.5*min(|d|,1); losers have
            # d == 0 exactly (pad cell, tv 0) so they contribute 0
            d = sb.tile([P, GC], f32)
            nc.vector.tensor_sub(d[:], gat[:], A[:, 0:GC])
            ad = sb.tile([P, GC], f32)
            nc.vector.scalar_tensor_tensor(ad[:], d[:], -1.0, d[:], OP.mult, OP.max)
            h = sb.tile([P, GC], f32)
            nc.vector.tensor_scalar(h[:], ad[:], 1.0, 0.5, OP.min, OP.mult)
            u = sb.tile([P, GC], f32)
            nc.vector.tensor_sub(u[:], ad[:], h[:])
            nc.vector.tensor_mul(Ab[:, 0:GC], h[:], u[:])  # le, bf16 on write

            # partition reduction: ones^T @ [le | wfull], single-pass bf16
            ps = pp.tile([1, 2 * GC], f32)
            nc.tensor.matmul(ps[:], ones[:], Ab[:, :])
            res = sb.tile([1, 2], f32)
            nc.vector.reduce_sum(
                res[:, :],
                ps[:, :].rearrange("p (g c) -> p g c", g=2),
                axis=AX.X,
            )
            nc.sync.dma_start(out_d[:, :], res[:])

    nc.compile()
    return nc


def _get_nc():
    if "nc" not in _cached:
        _cached["nc"] = _build_nc()
    return _cached["nc"]


def _make_in_maps(preds, targets):
    # grid cells exactly as the reference computes them (all-f32 arithmetic)
    five = np.float32(W / 80.0)
    gx = np.clip(np.floor(targets[..., 0] * five), 0, W - 1).astype(np.int64)
    gy = np.clip(np.floor(targets[..., 1] * five), 0, H - 1).astype(np.int64)
    cells = gy * W + gx                                  # [B, T]
    # last-writer-wins: target t loses if any t' > t hits the same cell
    eq = cells[:, :, None] == cells[:, None, :]          # [B, T, T]
    later = np.triu(np.ones((T, T), dtype=bool), k=1)
    win = ~np.any(eq & later, axis=2)                    # [B, T]

    # channels-last relayout so each target's 7 channels are one contiguous
    # 28B indirect-DMA row; 8 zero pad cells per shard catch the losers
    preds_t = np.ascontiguousarray(preds.transpose(0, 2, 3, 1))

    jj = (np.arange(P) // 64)[:, None]                   # [128, 1]
    gg = np.arange(NG)[None, :]                          # [1, 2]
    jloc = gg * 2 + jj                                   # local image index
    tt = (np.arange(P) % 64)[:, None]                    # target index

    in_maps = []
    for k in range(NCORES):
        pshard = np.zeros((NELEM + PAD, 1), dtype=np.float32)
        pshard[:NELEM, 0] = preds_t[k * BLOC : (k + 1) * BLOC].reshape(NELEM)

        jglob = k * BLOC + jloc                          # [128, 2] global image
        wsh = win[jglob, tt]                             # [128, 2]
        offs = np.where(
            wsh, cells[jglob, tt] * C + jloc * CHW, NELEM
        ).astype(np.int32)

        tvals = targets[jglob, tt, :] * wsh[..., None]   # [128, 2, 7]
        wfull = np.broadcast_to(
            wsh[..., None].astype(np.float32), (P, NG, C)
        )

        # sort the 256 descriptors by DRAM offset (the partial sums are
        # permutation-invariant): each SDMA engine then reads a contiguous
        # address range, tightening the gather drain
        flat_off = offs.reshape(P * NG)
        order = np.argsort(flat_off, kind="stable")
        offs = flat_off[order].reshape(P, NG)
        tv_s = tvals.reshape(P * NG, C)[order].reshape(P, GC)
        wf_s = np.ascontiguousarray(wfull).reshape(P * NG, C)[order].reshape(P, GC)

        fbits = np.concatenate([tv_s, wf_s], axis=1).astype(np.float32).view(np.int32)
        aux = np.concatenate([offs, fbits], axis=1)

        in_maps.append(
            {"preds_flat": pshard, "aux": np.ascontiguousarray(aux)}
        )
    return in_maps


def kernel(preds, targets):
    from concourse.bass_utils import run_bass_kernel_spmd

    preds = np.ascontiguousarray(np.asarray(preds), dtype=np.float32)
    targets = np.ascontiguousarray(np.asarray(targets), dtype=np.float32)
    assert preds.shape == (B, C, H, W) and targets.shape == (B, T, C)

    nc = _get_nc()
    in_maps = _make_in_maps(preds, targets)
    res = run_bass_kernel_spmd(nc, in_maps, list(range(NCORES)), trace=TRACE)
    _cached["last_results"] = res

    lsum = 0.0
    nsum = 0.0
    for k in range(NCORES):
        part = res.results[k]["out"].reshape(2)
        lsum += 2.0 * float(part[0])
        nsum += float(part[1]) / 7.0
    loss = np.float32(lsum / (nsum + 1e-6))
    return loss, np.float32(nsum)
